# revision 37
# baseline (speedup 1.0000x reference)
"""Trainium2 Bass kernel for CausalSelfAttention (d_model=2048, 16 heads, s=2048, b=2).

Sharding: data-parallel over batch (2) x tensor-parallel over heads (4 groups
of 4 heads) = 8 cores.  Each core: qkv projection for its 4 heads, RoPE,
causal attention, partial o_proj (row-parallel); host sums 4 partials/batch.

v3: qkv and o_proj matmuls run in fp8e4 DoubleRow perf mode (0.5 PE
cycles/row, 256-deep contraction per instruction).  Full precision is kept
with a 3-matmul hi/lo split: w*x ~= w_hi*x_hi + w_lo*x_hi + w_hi*x_lo, which
costs 0.75x the bf16 rows for ~bf16 accuracy (lo terms carry the operands'
fp8 rounding error; the dropped lo*lo term is ~0.1%).  x/w hi+lo parts are
split host-side (free); o_proj's attn hi/lo split costs 2 extra vector ops
per [128,512] tile.  Weights are pre-scaled (wq/wk x128, wv/wo x32) so hi
AND lo parts sit in fp8e4's normal range; scales cancel in the exp bias (qk)
and a host-side divide (v*wo -> /1024).

Attention core (scores, exp, AV) stays bf16: its operands are produced
on-chip, so fp8 conversion would cost more vector-engine time than the PE
time it saves.  Softmax denominator: all tiles (incl. masked diagonals) are
accumulated elementwise on DVE/Pool into f32 tiles that one f32r matmul per
(head, chunk) folds -- no per-tile PE den matmuls.

Output is stored bf16 (halves the out DMA); host sums partials in f64.

DMA notes: the DGE descriptor-gen on the issuing sequencer (SP.SEQ) is a
scarce serial resource (~5.5ns/descriptor).  x and w are host-pre-laid so
every load is one 4-8KB descriptor per partition (128/load instead of
1024), and output tiles are written in [128, 2, SC] pairs (one DMA per two
dm-tiles).  Issuing DMAs from the Act hwdge queue instead stalls exp
dispatch and loses ~13us -- keep everything on SP.  gpsimd cannot access
PSUM (BIR verifier rejects it), so psum evacuation alternates Act/DVE only.
In s4 (attn(3), no projection running) score tiles borrow psMM's two psum
banks for elasticity against Act exp bursts.
"""

import sys

import numpy as np

_TRN_REPO = "/opt/trn_rl_repo"
if _TRN_REPO not in sys.path:
    sys.path.insert(0, _TRN_REPO)

import ml_dtypes  # noqa: E402

import concourse.tile as tile  # noqa: E402
import concourse.mybir as mybir  # noqa: E402
from concourse import bacc, bass_utils  # noqa: E402

# Problem constants (hardcoded per the contract).
S = 2048          # sequence length
B = 2             # batch
DM = 2048         # d_model
NH = 16           # heads total
DH = 128          # head dim
ROPE_THETA = 10000.0

N_CORES = 8
TP = 4            # head-parallel groups
HPC = NH // TP    # heads per core = 4
DHC = HPC * DH    # head-dim per core = 512

SC = 512          # s-chunk
NSC = S // SC     # 4 chunks
KT = DM // 128    # contraction tiles for projections = 16
KP = KT // 2      # DoubleRow pairs = 8
HKT = KT // 2

W_SCALE_QK = 128.0   # wq/wk pre-scale (squared into exp's scale arg)
W_SCALE_V = 32.0     # wv pre-scale
W_SCALE_O = 32.0     # wo pre-scale; host divides output by 32*32
SCALE = 1.0 / float(np.sqrt(DH)) / (W_SCALE_QK * W_SCALE_QK)
EXP_SHIFT = -25.0  # softmax computed as exp(score*scale - 25); shift cancels

F32 = mybir.dt.float32
F32R = mybir.dt.float32r
BF16 = mybir.dt.bfloat16
FP8 = mybir.dt.float8e4
NPBF = ml_dtypes.bfloat16
NP8 = ml_dtypes.float8_e4m3
DR = mybir.MatmulPerfMode.DoubleRow

WARMUP_MMS = 34   # 512-row dummies spanning the startup DMA gate
# full-tile den accumulation: tiles [0, split) chain on DVE into da_a,
# [split, n_full) chain on Pool into da_b -- two single-engine chains so no
# cross-engine semaphore hops serialize the fold matmul.  Diagonal tiles
# alternate between the chains (qc=0 runs a single DVE chain).
DEN_SPLIT = {1: 2, 2: 4, 3: 8}

_CACHE = {}

# stage interleave fractions (tunable): (attn_qc, proj_sc, oproj_qc,
# prefetch, tail_frac) per stage is fixed; these are the tail fracs.
STAGE_TF = [0.3, 0.3, 0.35, 0.35]
# tuning knobs (bisectable)
K_DMA_SPLIT = False   # Act-queue DMA dispatch blocks exp dispatch: keep off
K_QC3_PSMM = True     # borrow psMM banks for qc3 score tiles
K_QC2_EVAC_ALT = 0  # qc2 osb evac: 0=vector, 1=alternate, 2=gpsimd
K_DEN3_DVE = False    # qc3 diag den adds mostly on DVE
K_CASCADE = False     # head-split attn cascade across stages


def _build_program(warmup=None, stage_tf=None, knobs=None):
    warmup = WARMUP_MMS if warmup is None else warmup
    stage_tf = STAGE_TF if stage_tf is None else stage_tf
    k_split = K_DMA_SPLIT if knobs is None else knobs.get('split', K_DMA_SPLIT)
    k_psmm = K_QC3_PSMM if knobs is None else knobs.get('psmm', K_QC3_PSMM)
    k_evac = K_QC2_EVAC_ALT if knobs is None else knobs.get('evac', K_QC2_EVAC_ALT)
    k_den3 = K_DEN3_DVE if knobs is None else knobs.get('den3', K_DEN3_DVE)
    k_casc = K_CASCADE if knobs is None else knobs.get('casc', K_CASCADE)
    _scalar_q = None  # set after nc created
    nc = bacc.Bacc("TRN2", target_bir_lowering=False, debug=False,
                   num_devices=N_CORES)

    # ---- I/O ----
    # x: host pre-laid as [sc, half, p, dk, s] so each (chunk, half) load is
    # one descriptor per partition (4KB contiguous) instead of 1024 x 512B --
    # the DGE descriptor-gen on the issuing sequencer is the scarce resource.
    x_hi_T = nc.dram_tensor("x_hi_T", [NSC * 2 * 128, HKT * SC], FP8,
                            kind="ExternalInput")
    x_lo_T = nc.dram_tensor("x_lo_T", [NSC * 2 * 128, HKT * SC], FP8,
                            kind="ExternalInput")
    # w: host pre-laid as [p, dk, c] (8KB contiguous per partition)
    wq_hi_T = nc.dram_tensor("wq_hi_T", [128, KT * DHC], FP8, kind="ExternalInput")
    wq_lo_T = nc.dram_tensor("wq_lo_T", [128, KT * DHC], FP8, kind="ExternalInput")
    wk_hi_T = nc.dram_tensor("wk_hi_T", [128, KT * DHC], FP8, kind="ExternalInput")
    wk_lo_T = nc.dram_tensor("wk_lo_T", [128, KT * DHC], FP8, kind="ExternalInput")
    wv_hi_T = nc.dram_tensor("wv_hi_T", [128, KT * DHC], FP8, kind="ExternalInput")
    wv_lo_T = nc.dram_tensor("wv_lo_T", [128, KT * DHC], FP8, kind="ExternalInput")
    # o_proj weights pre-packed as head pairs: [hp][128, 2, DM]
    wo_hi_T = nc.dram_tensor("wo_hi_T", [128, 2 * 2 * DM], FP8,
                             kind="ExternalInput")
    wo_lo_T = nc.dram_tensor("wo_lo_T", [128, 2 * 2 * DM], FP8,
                             kind="ExternalInput")
    cos_t = nc.dram_tensor("cos_t", [DH, S], BF16, kind="ExternalInput")
    sin_t = nc.dram_tensor("sin_t", [DH, S], BF16, kind="ExternalInput")
    mask_t = nc.dram_tensor("mask_t", [128, 128], BF16, kind="ExternalInput")
    ones_fr_t = nc.dram_tensor("ones_fr_t", [128, 1], F32R, kind="ExternalInput")
    out_T = nc.dram_tensor("out_T", [DM, S], BF16, kind="ExternalOutput")

    with tile.TileContext(nc) as tc:
      with (
          tc.tile_pool(name="wts", bufs=1) as wpool,      # weights + consts
          tc.tile_pool(name="qkv", bufs=1) as apool,      # q/k/v chunk tiles
          tc.tile_pool(name="xin", bufs=4) as xpool,      # x halves (hi+lo)
          tc.tile_pool(name="rope", bufs=2) as rpool,     # rope scratch
          tc.tile_pool(name="pt", bufs=8) as ppool,       # exp(p) tiles
          tc.tile_pool(name="an", bufs=4) as anpool,      # attn hi/lo pairs
          tc.tile_pool(name="af", bufs=2) as afpool,      # attn f32 scratch
          tc.tile_pool(name="da", bufs=2) as dapool,      # den accumulators
          tc.tile_pool(name="sm", bufs=2) as spool,       # recip / rbs
          tc.tile_pool(name="ob", bufs=4) as opool,       # o_proj staging
      ):
        # ---------------- persistent tiles ----------------
        # projection weights: [128, KT, DHC] fp8, pair-sliced [:, 2j:2j+2, :]
        wq_w = [wpool.tile([128, KT, DHC], FP8, tag=f"wq{v}", name=f"wq{v}")
                for v in ("h", "l")]
        wk_w = [wpool.tile([128, KT, DHC], FP8, tag=f"wk{v}", name=f"wk{v}")
                for v in ("h", "l")]
        wv_w = [wpool.tile([128, KT, DHC], FP8, tag=f"wv{v}", name=f"wv{v}")
                for v in ("h", "l")]
        # o_proj weights: per head pair hp: [128, 2, DM] fp8
        wo_t = [[wpool.tile([128, 2, DM], FP8, tag=f"wo{v}{hp}",
                            name=f"wo{v}{hp}") for hp in range(2)]
                for v in range(2)]
        cos_sb = wpool.tile([DH, S], BF16, tag="cos")
        sin_sb = wpool.tile([DH, S], BF16, tag="sin")
        mask_sb = wpool.tile([128, 128], BF16, tag="mask")
        ones_fr = wpool.tile([128, 1], F32R, tag="onesf")
        bias_sb = wpool.tile([128, 1], F32, tag="bias")
        wu_d = wpool.tile([128, 512], BF16, tag="wud")

        # q/k: [dh, s-chunk] per (head, chunk); v: [s-block, dhc] per block
        qt = [[apool.tile([DH, SC], BF16, tag=f"q{h}_{sc}", name=f"q{h}_{sc}")
               for sc in range(NSC)] for h in range(HPC)]
        kt_ = [[apool.tile([DH, SC], BF16, tag=f"k{h}_{sc}", name=f"k{h}_{sc}")
                for sc in range(NSC)] for h in range(HPC)]
        vblk = [apool.tile([128, DHC], BF16, tag=f"v{i}", name=f"v{i}")
                for i in range(S // 128)]

        # ---------------- PSUM pools (stages 0-4; s5 swaps to psOZ) --------
        _ps_ctx = [tc.tile_pool(name=n, bufs=b, space="PSUM")
                   for n, b in [("psMM", 2), ("psSC", 2), ("psAC", 2),
                                ("psDN", 1), ("psOP", 1)]]
        psMM, psSC, psAC, psDN, psOP = [p.__enter__() for p in _ps_ctx]

        # ---------------- startup ----------------
        scalar_q = nc.scalar if k_split else nc.sync
        nc.gpsimd.memset(wu_d[:], 0.0)
        nc.vector.memset(bias_sb[:], EXP_SHIFT)
        wu_ps = psSC.tile([128, SC], F32, tag="sc", name="wu_ps")

        def _warmup(n):
            for _ in range(n):
                nc.tensor.matmul(wu_ps[:, 0:SC], wu_d[:, 0:128],
                                 wu_d[:, 0:SC], start=True, stop=True)

        def _w_src(t, i0, n):
            return (t[:, i0 * DHC:(i0 + n) * DHC]
                    .rearrange("p (dk c) -> p dk c", c=DHC))

        def _load_x_half(sc, i, eng=None):
            """Load x chunk-half (hi+lo) as [128, HKT, SC] fp8 tiles."""
            xh = xpool.tile([128, HKT, SC], FP8, tag="xh", name=f"xh{sc}_{i}")
            xl = xpool.tile([128, HKT, SC], FP8, tag="xl", name=f"xl{sc}_{i}")
            row = (sc * 2 + i) * 128
            for n, (t, src) in enumerate(((xh, x_hi_T), (xl, x_lo_T))):
                e = eng if eng is not None else (nc.sync, scalar_q)[n]
                e.dma_start(t[:, :, :],
                            src[row:row + 128, :]
                            .rearrange("p (dk s) -> p dk s", s=SC))
            return (xh, xl)

        _warmup(warmup)
        # preload the Exp activation table off the critical path
        dummy = wpool.tile([128, 1], F32, tag="dumm")
        nc.scalar.activation(dummy[:], bias_sb[:],
                             mybir.ActivationFunctionType.Exp)

        nc.sync.dma_start(mask_sb[:], mask_t[:, :])
        nc.sync.dma_start(ones_fr[:], ones_fr_t[:, :])
        x_half = {}
        # chunk-0 loads split and interleaved x/w (sync + scalar queues) so
        # the first projection chain can start early
        xh0 = xpool.tile([128, HKT, SC], FP8, tag="xh", name="xh0_0")
        xl0 = xpool.tile([128, HKT, SC], FP8, tag="xl", name="xl0_0")
        xhd = xh0[:].rearrange("p dk s -> p (dk s)")
        xld = xl0[:].rearrange("p dk s -> p (dk s)")
        HB = HKT * SC // 2
        nc.sync.dma_start(xhd[:, 0:HB], x_hi_T[0:128, 0:HB])
        scalar_q.dma_start(wq_w[0][:, 0:4, :], _w_src(wq_hi_T, 0, 4))
        nc.sync.dma_start(xld[:, 0:HB], x_lo_T[0:128, 0:HB])
        scalar_q.dma_start(wq_w[1][:, 0:4, :], _w_src(wq_lo_T, 0, 4))
        nc.sync.dma_start(xhd[:, HB:2 * HB], x_hi_T[0:128, HB:2 * HB])
        scalar_q.dma_start(wq_w[0][:, 4:16, :], _w_src(wq_hi_T, 4, 12))
        nc.sync.dma_start(xld[:, HB:2 * HB], x_lo_T[0:128, HB:2 * HB])
        scalar_q.dma_start(wq_w[1][:, 4:16, :], _w_src(wq_lo_T, 4, 12))
        x_half[(0, 0)] = (xh0, xl0)
        x_half[(0, 1)] = _load_x_half(0, 1)
        nc.sync.dma_start(wk_w[0][:, :, :], _w_src(wk_hi_T, 0, KT))
        scalar_q.dma_start(wk_w[1][:, :, :], _w_src(wk_lo_T, 0, KT))
        nc.sync.dma_start(cos_sb[:], cos_t[:, :])
        scalar_q.dma_start(sin_sb[:], sin_t[:, :])
        nc.sync.dma_start(wv_w[0][:, :, :], _w_src(wv_hi_T, 0, KT))
        scalar_q.dma_start(wv_w[1][:, :, :], _w_src(wv_lo_T, 0, KT))
        x_half[(1, 0)] = _load_x_half(1, 0)
        x_half[(1, 1)] = _load_x_half(1, 1)
        for v, src in ((0, wo_hi_T), (1, wo_lo_T)):
            for hp in range(2):
                (nc.sync, scalar_q)[hp].dma_start(
                    wo_t[v][hp][:, :, :],
                    src[:, hp * 2 * DM:(hp + 1) * 2 * DM]
                    .rearrange("p (two m) -> p two m", two=2))

        # ---------------- op generators ----------------
        def proj_fillers(sc):
            """Yield closures, one per PE matmul, for projection chunk sc.
            Each chain: 8 k-pairs x 3 hi/lo terms of fp8 DoubleRow matmuls.
            Chain-end closures also emit the evac/rope bundle."""
            ssl = slice(sc * SC, (sc + 1) * SC)
            chains = []
            for which, _ in (("q", None), ("k", None)):
                for h in range(HPC):
                    chains.append((which, h))
            for st in range(SC // 128):
                chains.append(("v", st))

            def xpair(j, v):
                xh, xl = x_half[(sc, j // 4)]
                t = (xh, xl)[v]
                jj = j % 4
                return t[:, 2 * jj:2 * jj + 2, :]

            for which, idx in chains:
                hold = {}
                n_mm = 3 * KP
                for mi in range(n_mm):
                    j, term = divmod(mi, 3)

                    def mm(hold=hold, j=j, term=term, which=which, idx=idx,
                           sc=sc, mi=mi):
                        if mi == 0:
                            hold["ps"] = psMM.tile(
                                [128, SC], F32, tag="mm",
                                name=f"mm_{which}{idx}_{sc}")
                        ps = hold["ps"]
                        # term 0: w_hi*x_hi, 1: w_lo*x_hi, 2: w_hi*x_lo
                        wv_ = 1 if term == 1 else 0
                        xv_ = 1 if term == 2 else 0
                        st_ = (mi == 0)
                        sp_ = (mi == n_mm - 1)
                        if which == "v":
                            nc.tensor.matmul(
                                ps[:],
                                xpair(j, xv_)[:, :, idx * 128:(idx + 1) * 128],
                                wv_w[wv_][:, 2 * j:2 * j + 2, :],
                                start=st_, stop=sp_, perf_mode=DR)
                        else:
                            ww = (wq_w if which == "q" else wk_w)[wv_]
                            hsl = slice(idx * DH, (idx + 1) * DH)
                            nc.tensor.matmul(
                                ps[:], ww[:, 2 * j:2 * j + 2, hsl],
                                xpair(j, xv_), start=st_, stop=sp_,
                                perf_mode=DR)
                    if mi < n_mm - 1:
                        yield mm
                        continue

                    def tail(mm=mm, hold=hold, which=which, idx=idx, sc=sc,
                             ssl=ssl):
                        mm()
                        ps = hold["ps"]
                        if which == "v":
                            nc.vector.tensor_copy(vblk[sc * 4 + idx][:], ps[:])
                            return
                        raw = rpool.tile([128, SC], F32, tag="raw")
                        nc.scalar.copy(raw[:], ps[:])
                        qsw = rpool.tile([128, SC], F32, tag="qsw")
                        nc.vector.stream_shuffle(
                            qsw[:], raw[:],
                            mask=list(range(16, 32)) + list(range(0, 16)))
                        nc.vector.tensor_mul(qsw[:], qsw[:], sin_sb[:, ssl])
                        t1 = rpool.tile([128, SC], BF16, tag="t1")
                        nc.gpsimd.tensor_mul(t1[:], raw[:], cos_sb[:, ssl])
                        dst = (qt if which == "q" else kt_)[idx][sc]
                        nc.vector.tensor_add(dst[:], qsw[:], t1[:])
                    yield tail

        def attn_steps(qc, heads=None):
            """Yield closures for attention of `heads` at query chunk qc.
            Each step: score(kt) + exp + mask/den work + av(kt-3)."""
            n_kt = 4 * qc + 4
            n_full = n_kt - 4
            for h in (range(HPC) if heads is None else heads):
                u = {"pts": [None] * n_kt}

                def av(kt, h=h, u=u, n_kt=n_kt, qc=qc):
                    off = kt * 128 - qc * SC
                    q_lo = max(0, off)
                    ln = SC - q_lo
                    nc.tensor.matmul(
                        u["acc"][:, q_lo:SC],
                        vblk[kt][:, h * DH:(h + 1) * DH],
                        u["pts"][kt][:, 0:ln],
                        start=(kt == 0), stop=(kt == n_kt - 1))

                for kt in range(n_kt):
                    def step(kt=kt, h=h, u=u, n_kt=n_kt, n_full=n_full,
                             qc=qc, av=av):
                        if kt == 0:
                            u["acc"] = psAC.tile([128, SC], F32, tag="ac",
                                                 name=f"ac{h}_{qc}")
                        off = kt * 128 - qc * SC
                        q_lo = max(0, off)
                        ln = SC - q_lo
                        # qc3 (s4, no proj running): borrow psMM's two banks
                        # for extra score elasticity against Act exp bursts
                        if k_psmm and qc == 3 and kt % 2 == 1:
                            sp = psMM.tile([128, SC], F32, tag="mm",
                                           name=f"sp{h}_{qc}_{kt}")
                        else:
                            sp = psSC.tile([128, SC], F32, tag="sc",
                                           name=f"sp{h}_{qc}_{kt}")
                        nc.tensor.matmul(
                            sp[:, 0:ln], kt_[h][kt // 4][:, (kt % 4) * 128:
                                                         (kt % 4 + 1) * 128],
                            qt[h][qc][:, q_lo:SC], start=True, stop=True)
                        if kt >= 3:
                            av(kt - 3)
                        pt = ppool.tile([128, SC], BF16, tag="pt",
                                        name=f"pt{h}_{qc}_{kt}")
                        u["pts"][kt] = pt
                        nc.scalar.activation(
                            pt[:, 0:ln], sp[:, 0:ln],
                            mybir.ActivationFunctionType.Exp,
                            bias=bias_sb[:], scale=SCALE)
                        # den accumulator chains: full tiles split between a
                        # DVE chain (daa) and a Pool chain (dab); masked
                        # diagonal tiles alternate between them at offsets.
                        if off >= 0:
                            nc.vector.tensor_mul(
                                pt[:, 0:128], pt[:, 0:128], mask_sb[:, :])
                            di = kt - n_full
                            if qc == 0:
                                if kt == 0:
                                    u["daa"] = dapool.tile(
                                        [128, SC], F32R, tag="daa",
                                        name=f"daa{h}_{qc}")
                                    nc.vector.tensor_copy(u["daa"][:], pt[:])
                                else:
                                    nc.vector.tensor_add(
                                        u["daa"][:, q_lo:SC],
                                        u["daa"][:, q_lo:SC], pt[:, 0:ln])
                            else:
                                # qc3: keep the Pool chain short (it trails
                                # into fin and stalls the den fold)
                                dve_ = (di < 3) if (qc == 3 and k_den3) else (di % 2 == 0)
                                grp, eng = (("daa", nc.vector) if dve_
                                            else ("dab", nc.gpsimd))
                                eng.tensor_add(u[grp][:, q_lo:SC],
                                               u[grp][:, q_lo:SC],
                                               pt[:, 0:ln])
                        else:
                            split = DEN_SPLIT[qc]
                            grp, eng, i0 = (
                                ("daa", nc.vector, 0) if kt < split
                                else ("dab", nc.gpsimd, split))
                            if kt == i0:
                                pass  # init pairs with the next tile
                            elif kt == i0 + 1:
                                u[grp] = dapool.tile([128, SC], F32R, tag=grp,
                                                     name=f"{grp}{h}_{qc}")
                                eng.tensor_add(u[grp][:],
                                               u["pts"][i0][:], pt[:])
                            else:
                                eng.tensor_add(u[grp][:], u[grp][:], pt[:])
                    yield step

                def fin(h=h, qc=qc, u=u, n_kt=n_kt, av=av):
                    av(n_kt - 3)
                    av(n_kt - 2)
                    av(n_kt - 1)
                    den = psDN.tile([1, SC], F32, tag="dn", name=f"dn{h}_{qc}")
                    groups = [g for g in ("daa", "dab") if g in u]
                    for gi, grp in enumerate(groups):
                        nc.tensor.matmul(
                            den[:, 0:SC], ones_fr[:], u[grp][:],
                            start=(gi == 0), stop=(gi == len(groups) - 1))
                    recipf = spool.tile([1, SC], F32, tag="recipf")
                    nc.vector.reciprocal_approx_fast(out=recipf[:],
                                                     in_=den[:])
                    rbs = spool.tile([128, SC], F32, tag="rbs")
                    nc.gpsimd.partition_broadcast(rbs[:], recipf[:])
                    # attn normalize + hi/lo fp8 split into the pair tile
                    hp, hi_ = divmod(h, 2)
                    if hi_ == 0:
                        attn_hi[hp][qc] = anpool.tile(
                            [128, 2, SC], FP8, tag="anh", name=f"anh{hp}_{qc}")
                        attn_lo[hp][qc] = anpool.tile(
                            [128, 2, SC], FP8, tag="anl", name=f"anl{hp}_{qc}")
                    af = afpool.tile([DH, SC], F32, tag="af",
                                     name=f"af{h}_{qc}")
                    nc.vector.tensor_mul(af[:], u["acc"][:], rbs[:])
                    ah = attn_hi[hp][qc][:, hi_, :]
                    nc.vector.tensor_copy(ah, af[:])
                    nc.gpsimd.tensor_sub(attn_lo[hp][qc][:, hi_, :],
                                         af[:], ah)
                yield fin

        attn_hi = [[None] * NSC for _ in range(2)]
        attn_lo = [[None] * NSC for _ in range(2)]

        def oproj_fillers(qc, pspool=None, pstag="op"):
            """Yield per-matmul closures for o_proj of chunk qc: per dm tile,
            2 head pairs x 3 hi/lo terms of fp8 DoubleRow matmuls."""
            if pspool is None:
                pspool = psOP
            qsl = slice(qc * SC, (qc + 1) * SC)
            pair = {}
            for mt in range(DM // 128):
                msl = slice(mt * 128, (mt + 1) * 128)
                hold = {}
                n_mm = 6
                for mi in range(n_mm):
                    hp, term = divmod(mi, 3)

                    def mm(hp=hp, term=term, hold=hold, msl=msl, qc=qc, mt=mt,
                           mi=mi, pspool=pspool, pstag=pstag):
                        if mi == 0:
                            hold["ops"] = pspool.tile([128, SC], F32,
                                                      tag=pstag,
                                                      name=f"ops{mt}_{qc}")
                        wv_ = 1 if term == 1 else 0
                        av_ = 1 if term == 2 else 0
                        an = (attn_hi, attn_lo)[av_][hp][qc]
                        nc.tensor.matmul(
                            hold["ops"][:], wo_t[wv_][hp][:, :, msl],
                            an[:, :, :], start=(mi == 0),
                            stop=(mi == n_mm - 1), perf_mode=DR)
                    if mi < n_mm - 1:
                        yield mm
                        continue

                    def tail(mm=mm, hold=hold, mt=mt, qc=qc, msl=msl,
                             qsl=qsl, pair=pair):
                        mm()
                        if mt % 2 == 0:
                            pair["t"] = opool.tile([128, 2, SC], BF16,
                                                   tag="ob",
                                                   name=f"osb{mt}_{qc}")
                        osb = pair["t"][:, mt % 2, :]
                        # NOTE: gpsimd cannot access PSUM (BIR verifier)
                        if qc == 3:   # s5: alternate so no engine binds
                            (nc.scalar.copy,
                             nc.vector.tensor_copy)[mt % 2](
                                osb, hold["ops"][:])
                        elif qc != 2:
                            nc.scalar.copy(osb, hold["ops"][:])
                        else:  # Act is exp-loaded mid-s4
                            nc.vector.tensor_copy(osb, hold["ops"][:])
                        if mt % 2 == 1:
                            m2 = slice((mt - 1) * 128, (mt + 1) * 128)
                            nc.sync.dma_start(
                                out_T[m2, qsl]
                                .rearrange("(two p) s -> p two s", p=128),
                                pair["t"][:, :, :])
                    yield tail

        # ---------------- interleaved emission ----------------
        def emit_stage(attn_qc, proj_sc, oproj_qc, prefetch_sc,
                       tail_frac=0.0):
            """Emit one pipeline stage.  Attention steps are spread through
            the first (1-tail_frac) of the filler list so their finalize
            chains drain under the remaining fillers."""
            fillers = []
            if prefetch_sc is not None and prefetch_sc < NSC:
                def pf(prefetch_sc=prefetch_sc):
                    x_half[(prefetch_sc, 0)] = _load_x_half(prefetch_sc, 0)
                    x_half[(prefetch_sc, 1)] = _load_x_half(prefetch_sc, 1)
                fillers.append(pf)
            if proj_sc is not None:
                fillers.extend(proj_fillers(proj_sc))
            if oproj_qc is not None:
                fillers.extend(oproj_fillers(oproj_qc))
            steps = (list(attn_steps(*attn_qc))
                     if attn_qc is not None else [])
            if not steps:
                for f in fillers:
                    f()
                return
            nf, ns = len(fillers), len(steps)
            spread_n = min(nf, max(ns, int(nf * (1.0 - tail_frac))))
            fi = 0
            for si, st in enumerate(steps):
                st()
                target = (si + 1) * spread_n // ns
                while fi < target:
                    fillers[fi]()
                    fi += 1
            while fi < nf:
                fillers[fi]()
                fi += 1

        emit_stage(None, 0, None, 2)               # s0: proj(0), prefetch x2
        if k_casc:
            # head-split cascade: each attn (qc, head-pair) lands in a
            # stage whose Act exp load fits under the stage's PE time.
            emit_stage((0, None), 1, None, 3, stage_tf[0])
            emit_stage((1, None), 2, None, None, stage_tf[1])
            emit_stage((2, [0, 1]), None, 0, None, 0.0)
            emit_stage((2, [2, 3]), 3, None, None, stage_tf[2])
            emit_stage((3, [0, 1]), None, 1, None, 0.0)
            emit_stage((3, [2, 3]), None, 2, None, stage_tf[3])
        else:
            emit_stage((0, None), 1, None, 3, stage_tf[0])
            emit_stage((1, None), 2, 0, None, stage_tf[1])
            emit_stage((2, None), 3, 1, None, stage_tf[2])
            emit_stage((3, None), None, 2, None, stage_tf[3])

        # s5: o_proj(3) with a deeper psum rotation (other pools closed)
        for p in reversed(_ps_ctx):
            p.__exit__(None, None, None)
        with tc.tile_pool(name="psOZ", bufs=6, space="PSUM") as psOZ:
            for f in oproj_fillers(3, pspool=psOZ, pstag="oz"):
                f()

    nc.compile()
    return nc


def _split8(a):
    hi = a.astype(NP8)
    lo = (a - hi.astype(np.float32)).astype(NP8)
    return hi, lo


def _host_inputs(hidden_states, qkv_w, o_w):
    """Build the 8 per-core input maps (sharding + layout transforms)."""
    # RoPE pair i=16b+j lands at partitions 32b+j (even) and 32b+16+j (odd):
    # the pair swap is a within-32-block 16-rotation (one DVE stream_shuffle),
    # with the sign carried by the sin table.
    inv_freq = 1.0 / (ROPE_THETA ** (np.arange(0, DH, 2, dtype=np.float32) / DH))
    t = np.arange(S, dtype=np.float32)
    ang = np.outer(inv_freq, t)                       # [64, S]
    cosv, sinv = np.cos(ang), np.sin(ang)
    cos_t = np.zeros((DH, S), dtype=np.float32)
    sin_t = np.zeros((DH, S), dtype=np.float32)
    perm = np.zeros(DH, dtype=np.int64)
    for b in range(4):
        for j in range(16):
            i = 16 * b + j
            perm[32 * b + j] = 2 * i
            perm[32 * b + 16 + j] = 2 * i + 1
            cos_t[32 * b + j] = cosv[i]
            cos_t[32 * b + 16 + j] = cosv[i]
            sin_t[32 * b + j] = -sinv[i]
            sin_t[32 * b + 16 + j] = sinv[i]
    cos_t = cos_t.astype(NPBF)
    sin_t = sin_t.astype(NPBF)
    hperm = np.concatenate([g * DH + perm for g in range(HPC)])

    # mask128[p, j] = 1 if j >= p (valid) else 0, for diagonal score tiles
    mask128 = (np.arange(128)[None, :] >= np.arange(128)[:, None])
    mask128 = mask128.astype(NPBF)
    ones_fr = np.ones((128, 1), dtype=np.float32)

    def _x_layout(x_T):
        # [DM, S] -> [sc, half, p, dk, s] flattened to [NSC*2*128, HKT*SC]
        xr = x_T.reshape(2, HKT, 128, NSC, SC)      # [half, dk, p, sc, s]
        return np.ascontiguousarray(
            xr.transpose(3, 0, 2, 1, 4).reshape(NSC * 2 * 128, HKT * SC))

    def _w_layout(w_T):
        # [DM, DHC] -> [p, dk, c] flattened to [128, KT*DHC]
        return np.ascontiguousarray(
            w_T.reshape(KT, 128, DHC).transpose(1, 0, 2).reshape(128, -1))

    # x transposed + fp8 hi/lo split, shared per batch
    x_splits = []
    for b in range(B):
        x_T = np.ascontiguousarray(hidden_states[:, b, :].T)
        hi, lo = _split8(x_T)
        x_splits.append((_x_layout(hi), _x_layout(lo)))

    in_maps = []
    for c in range(N_CORES):
        b = c // TP
        g = c % TP
        hs = slice(g * DHC, (g + 1) * DHC)
        x_hi, x_lo = x_splits[b]
        wq = np.ascontiguousarray(
            qkv_w[0 * DM:1 * DM][hs][hperm].T) * W_SCALE_QK
        wk = np.ascontiguousarray(
            qkv_w[1 * DM:2 * DM][hs][hperm].T) * W_SCALE_QK
        wv = np.ascontiguousarray(qkv_w[2 * DM:3 * DM][hs].T) * W_SCALE_V
        wq_hi, wq_lo = [_w_layout(w) for w in _split8(wq)]
        wk_hi, wk_lo = [_w_layout(w) for w in _split8(wk)]
        wv_hi, wv_lo = [_w_layout(w) for w in _split8(wv)]
        # wo pair layout: [128, 2(hp-slot), 2, DM] -> [128, hp*2*DM + i*DM + m]
        wo = o_w[:, hs].T * W_SCALE_O                     # [DHC, DM]
        wo4 = wo.reshape(2, 2, 128, DM).transpose(2, 0, 1, 3)  # [128,hp,2,DM]
        wo_hi, wo_lo = _split8(np.ascontiguousarray(wo4.reshape(128, -1)))
        in_maps.append({
            "x_hi_T": x_hi, "x_lo_T": x_lo,
            "wq_hi_T": wq_hi, "wq_lo_T": wq_lo,
            "wk_hi_T": wk_hi, "wk_lo_T": wk_lo,
            "wv_hi_T": wv_hi, "wv_lo_T": wv_lo,
            "wo_hi_T": wo_hi, "wo_lo_T": wo_lo,
            "cos_t": cos_t, "sin_t": sin_t,
            "mask_t": mask128, "ones_fr_t": ones_fr,
        })
    return in_maps


def kernel(hidden_states, sequence_mask, qkv_w, o_w, _results_hook=None):
    hidden_states = np.asarray(hidden_states, dtype=np.float32)
    qkv_w = np.asarray(qkv_w, dtype=np.float32)
    o_w = np.asarray(o_w, dtype=np.float32)
    # sequence_mask is all-True for this problem shape (spec fill=ones).

    if "nc" not in _CACHE:
        _CACHE["nc"] = _build_program()
    nc = _CACHE["nc"]

    in_maps = _host_inputs(hidden_states, qkv_w, o_w)
    res = bass_utils.run_bass_kernel_spmd(
        nc, in_maps, core_ids=list(range(N_CORES)), trace=False)
    if _results_hook is not None:
        _results_hook(res)

    inv_scale = 1.0 / (W_SCALE_V * W_SCALE_O)
    out = np.zeros((S, B, DM), dtype=np.float64)
    for c in range(N_CORES):
        b = c // TP
        out[:, b, :] += res.results[c]["out_T"].T.astype(np.float64)
    return (out * inv_scale).astype(np.float32)


if __name__ == "__main__":
    rng = np.random.default_rng(0)
    hs = rng.standard_normal((S, B, DM), dtype=np.float32)
    sm = np.ones((B, S), dtype=bool)
    qw = (rng.standard_normal((3 * DM, DM), dtype=np.float32) * 0.02)
    ow = (rng.standard_normal((DM, DM), dtype=np.float32) * 0.02)
    o = kernel(hs, sm, qw, ow)
    print("out", o.shape, o.dtype, float(np.abs(o).mean()))
